# revision 10
# baseline (speedup 1.0000x reference)
"""Trainium2 Bass kernel for a 4-layer transformer decoder (self+cross attn, FFN,
BatchNorm, vocab projection).

Sharding (hardcoded): 8 cores; core c handles batch b=c//2 and vocab half
v=c%2.  The 4-layer trunk is replicated within each core pair (batch-parallel
x4); the vocab projection is split column-wise x2 within the pair.  No
collectives.

Device-side layout: activations live in SBUF transposed as [feature, token]
(4 tiles of [128, 1024]).  All matmuls run in bf16 with fp32 PSUM
accumulation.  BatchNorm (inference affine) and the Wo/W2 output biases are
folded on the host into per-feature scale/shift vectors.  Softmax is computed
without max-subtraction (score magnitudes are small); row sums are obtained by
appending a ones-column to the V operand of the P@V matmul.
"""

import sys

for _p in ("/opt/trn_rl_repo", "/root/.axon_site/_ro/trn_rl_repo"):
    if _p not in sys.path:
        sys.path.append(_p)

import numpy as np
import ml_dtypes

import concourse.bass as bass
import concourse.mybir as mybir
import concourse.tile as tile
from concourse import bacc, bass_utils
from concourse.bass_interp import get_hw_module

FP32 = mybir.dt.float32
BF16 = mybir.dt.bfloat16
AF = mybir.ActivationFunctionType
ALU = mybir.AluOpType

V, D, L, H, S, SE, B = 32000, 512, 4, 8, 1024, 1024, 4
DH = D // H          # 64
NCORES = 8
VH = V // 2          # vocab half per core
EPS = 1e-3
KT = D // 128        # 4 k-tiles over D
FT = (4 * D) // 128  # 16 f-tiles over the FFN hidden dim
SC = S // 128        # 8 token chunks of 128
NJ = S // 512        # 2 token chunks of 512
VC = VH // 500       # 32 vocab chunks of 500

# consts column map (per layer, CL columns per layer)
CL = 64
C_BQ_S, C_BK_S, C_BV_S = 0, 4, 8
C_BQ_X, C_BK_X, C_BV_X = 12, 16, 20
C_B1 = 24
C_BNB_S, C_BNB_T = 40, 44
C_BNM_S, C_BNM_T = 48, 52
C_BNF_S, C_BNF_T = 56, 60

_COMPILED = None  # (nc_hw, meta)


def _bf(x):
    return np.ascontiguousarray(np.asarray(x, np.float32)).astype(ml_dtypes.bfloat16)


def _pack_ktiles(w):
    """[K, F] -> [128, (K//128)*F] with k-tile-major column blocks."""
    k = w.shape[0] // 128
    return np.concatenate([w[i * 128:(i + 1) * 128] for i in range(k)], axis=1)


def _build_program():
    nc = bacc.Bacc("TRN2", target_bir_lowering=False, debug=False,
                   enable_asserts=False, num_devices=NCORES)

    x0T_d = nc.dram_tensor("x0T", [D, S], FP32, kind="ExternalInput").ap()
    encT_d = nc.dram_tensor("encT", [D, SE], BF16, kind="ExternalInput").ap()
    wself_d = nc.dram_tensor("wself", [L, 128, 4 * 2048], BF16, kind="ExternalInput").ap()
    wcross_d = nc.dram_tensor("wcross", [L, 128, 4 * 2048], BF16, kind="ExternalInput").ap()
    wffn1_d = nc.dram_tensor("wffn1", [L, 128, 4 * 2048], BF16, kind="ExternalInput").ap()
    wffn2_d = nc.dram_tensor("wffn2", [L, 128, 16 * 512], BF16, kind="ExternalInput").ap()
    wout_d = nc.dram_tensor("wout", [128, VC * 2000], BF16, kind="ExternalInput").ap()
    consts_d = nc.dram_tensor("consts", [128, L * CL], FP32, kind="ExternalInput").ap()
    masks_d = nc.dram_tensor("masks", [128, 4 * 512], BF16, kind="ExternalInput").ap()
    bvb_d = nc.dram_tensor("bvb", [128, 2 * L * 512], BF16, kind="ExternalInput").ap()
    logits_d = nc.dram_tensor("logits", [S, VH], FP32, kind="ExternalOutput").ap()

    with tile.TileContext(nc) as tc:
        _emit(nc, tc, x0T_d, encT_d, wself_d, wcross_d, wffn1_d, wffn2_d,
              wout_d, consts_d, masks_d, bvb_d, logits_d)

    nc.compile()
    nc.m = get_hw_module(nc.m)
    return nc


def _emit(nc, tc, x0T_d, encT_d, wself_d, wcross_d, wffn1_d, wffn2_d,
          wout_d, consts_d, masks_d, bvb_d, logits_d):
    import contextlib
    ctx = contextlib.ExitStack()
    with ctx:
        persist = ctx.enter_context(tc.tile_pool(name="persist", bufs=1))
        wpool = ctx.enter_context(tc.tile_pool(name="wpool", bufs=3))
        scr = ctx.enter_context(tc.tile_pool(name="scr", bufs=20))
        smalls = ctx.enter_context(tc.tile_pool(name="smalls", bufs=2))
        outp = ctx.enter_context(tc.tile_pool(name="outp", bufs=6))
        wog = ctx.enter_context(tc.tile_pool(name="wog", bufs=5))
        ps_mm = ctx.enter_context(tc.tile_pool(name="ps_mm", bufs=2, space="PSUM"))
        ps_sc = ctx.enter_context(tc.tile_pool(name="ps_sc", bufs=2, space="PSUM"))
        ps_ap = ctx.enter_context(tc.tile_pool(name="ps_ap", bufs=2, space="PSUM"))

        # persistent tiles
        xt = [persist.tile([128, S], FP32, name=f"xt{m}", tag=f"xt{m}") for m in range(KT)]
        xbf = [persist.tile([128, S], BF16, name=f"xbf{m}", tag=f"xbf{m}") for m in range(KT)]
        encbf = [persist.tile([128, SE], BF16, name=f"encbf{m}", tag=f"encbf{m}") for m in range(KT)]
        qt = [persist.tile([128, S], BF16, name=f"qt{m}", tag=f"qt{m}") for m in range(KT)]
        kt = [persist.tile([128, SE], BF16, name=f"kt{m}", tag=f"kt{m}") for m in range(KT)]
        att = [persist.tile([128, S], BF16, name=f"att{m}", tag=f"att{m}") for m in range(KT)]
        vt = [persist.tile([128, 8 * 65], BF16, name=f"vt{t}", tag=f"vt{t}") for t in range(SC)]
        consts = persist.tile([128, L * CL], FP32, name="consts", tag="consts")
        maskt = persist.tile([128, 4 * 512], BF16, name="maskt", tag="maskt")
        bvb = persist.tile([128, 2 * L * 512], BF16, name="bvb", tag="bvb")
        ones1 = persist.tile([1, 64], FP32, name="ones1", tag="ones1")

        nc.sync.dma_start(consts[:], consts_d[:])
        nc.sync.dma_start(maskt[:], masks_d[:])
        nc.sync.dma_start(bvb[:], bvb_d[:])
        nc.vector.memset(ones1[:], 1.0)
        for m in range(KT):
            nc.sync.dma_start(xt[m][:], x0T_d[m * 128:(m + 1) * 128, :])
            nc.sync.dma_start(encbf[m][:], encT_d[m * 128:(m + 1) * 128, :])
            nc.scalar.activation(xbf[m][:], xt[m][:], AF.Copy)

        def cc(i, base, m):
            """consts column AP [128,1] for layer i."""
            return consts[:, i * CL + base + m : i * CL + base + m + 1]

        def proj_T(wp, widx, src, dst, bias_base, i):
            """dst[fm][:, :] (bf16, transposed layout) = W.T @ src + bias.
            wp: weight pack tile; widx: which 512-col weight in the pack;
            src: list of 4 bf16 [128, S] tiles."""
            for m in range(KT):
                for j in range(NJ):
                    ps = ps_mm.tile([128, 512], FP32, name="ps", tag="mm")
                    for k in range(KT):
                        nc.tensor.matmul(
                            ps[:],
                            wp[:, k * 2048 + widx * 512 + m * 128:
                               k * 2048 + widx * 512 + (m + 1) * 128],
                            src[k][:, j * 512:(j + 1) * 512],
                            start=(k == 0), stop=(k == KT - 1))
                    nc.vector.tensor_scalar(
                        dst[m][:, j * 512:(j + 1) * 512], ps[:],
                        cc(i, bias_base, m), None, ALU.add)

        def v_natural(wp, src, bv_blk):
            """vt[sc] heads-of-65 layout (bf16) = (src.T @ Wv + bv) per 128-token
            chunk, plus ones columns.  bv is folded here: P@(V+bv) = N + d*bv, so
            after the divide the attention output already carries +bv."""
            bv3 = bvb[:, bv_blk * 512:(bv_blk + 1) * 512].rearrange(
                "p (h c) -> p h c", h=H)
            for sc_i in range(SC):
                ps = ps_mm.tile([128, 512], FP32, name="ps", tag="mm")
                for k in range(KT):
                    nc.tensor.matmul(
                        ps[:],
                        src[k][:, sc_i * 128:(sc_i + 1) * 128],
                        wp[:, k * 2048 + 2 * 512:k * 2048 + 3 * 512],
                        start=(k == 0), stop=(k == KT - 1))
                v3 = vt[sc_i].rearrange("p (h c) -> p h c", h=H)
                p3 = ps.rearrange("p (h c) -> p h c", h=H)
                nc.vector.tensor_tensor(v3[:, :, 0:64], p3[:], bv3[:], ALU.add)
                nc.vector.memset(v3[:, :, 64:65], 1.0)

        def attn_core(i, causal):
            """scores/softmax/apply; writes att (bf16 transposed).  Heads are
            processed in pairs with adjacent score matmuls on partition bases
            0/64 so the PE packs them into disjoint row groups."""
            n_kt = SC
            for pr in range(H // 2):
                pt = {}   # (hh, t) -> P^T tile, hh in {0, 1}
                for t in range(n_kt):
                    js = [0, 1] if (not causal or t < 4) else [1]
                    psc = [ps_sc.tile([128, 1024], FP32, name=f"psc{hh}", tag="sc")
                           for hh in range(2)]
                    for j in js:
                        for hh in range(2):
                            hp = 64 * hh
                            nc.tensor.matmul(
                                psc[hh][:, j * 512:(j + 1) * 512],
                                kt[pr][hp:hp + 64, t * 128:(t + 1) * 128],
                                qt[pr][hp:hp + 64, j * 512:(j + 1) * 512],
                                start=True, stop=True)
                    for hh in range(2):
                        p = scr.tile([128, 1024], BF16, name=f"pt{hh}", tag="big")
                        pt[(hh, t)] = p
                        if len(js) == 2:
                            nc.scalar.activation(p[:], psc[hh][:], AF.Exp)
                        else:
                            nc.scalar.activation(p[:, 512:1024], psc[hh][:, 512:1024],
                                                 AF.Exp)
                        if causal:
                            r = t if t < 4 else t - 4
                            sl = slice(0, 512) if t < 4 else slice(512, 1024)
                            nc.vector.tensor_tensor(
                                p[:, sl], p[:, sl],
                                maskt[:, r * 512:(r + 1) * 512], ALU.mult)
                for hh in range(2):
                    h = 2 * pr + hh
                    hp = 64 * hh
                    for j in range(NJ):
                        ts_list = list(range(4)) if (causal and j == 0) else list(range(n_kt))
                        aps = ps_ap.tile([65, 512], FP32, name="aps", tag="ap")
                        for n, t in enumerate(ts_list):
                            nc.tensor.matmul(
                                aps[:],
                                vt[t][:, 65 * h:65 * h + 65],
                                pt[(hh, t)][:, j * 512:(j + 1) * 512],
                                start=(n == 0), stop=(n == len(ts_list) - 1))
                        if causal and j == 0:
                            nc.vector.memset(aps[64:65, 0:1], 1.0)
                        rc = smalls.tile([1, 512], FP32, name="rc", tag="rc")
                        nc.vector.reciprocal(rc[:], aps[64:65, :])
                        bcp = ps_mm.tile([64, 512], FP32, name="bcp", tag="mm")
                        nc.tensor.matmul(bcp[:], ones1[:, 0:64], rc[:],
                                         start=True, stop=True)
                        rb = smalls.tile([64, 512], FP32, name="rb", tag="rb")
                        nc.scalar.activation(rb[:], bcp[:], AF.Copy)
                        dst = att[pr][hp:hp + 64, j * 512:(j + 1) * 512]
                        nc.vector.tensor_tensor(dst, aps[0:64, :], rb[:], ALU.mult)

        def o_proj_bn(i, wp, s_base, t_base):
            """x = (x + Wo.T@att) * s + t'; refresh xbf."""
            for m in range(KT):
                for j in range(NJ):
                    ps = ps_mm.tile([128, 512], FP32, name="ps", tag="mm")
                    for k in range(KT):
                        nc.tensor.matmul(
                            ps[:],
                            wp[:, k * 2048 + 3 * 512 + m * 128:
                               k * 2048 + 3 * 512 + (m + 1) * 128],
                            att[k][:, j * 512:(j + 1) * 512],
                            start=(k == 0), stop=(k == KT - 1))
                    xs = xt[m][:, j * 512:(j + 1) * 512]
                    nc.vector.tensor_add(xs, xs, ps[:])
                    nc.vector.tensor_scalar(xs, xs, cc(i, s_base, m),
                                            cc(i, t_base, m), ALU.mult, ALU.add)
                    nc.scalar.activation(xbf[m][:, j * 512:(j + 1) * 512], xs, AF.Copy)

        for i in range(L):
            # ---- self attention ----
            wp = wpool.tile([128, 4 * 2048], BF16, name="wp", tag="wpack")
            nc.sync.dma_start(wp[:], wself_d[i])
            proj_T(wp, 0, xbf, qt, C_BQ_S, i)
            proj_T(wp, 1, xbf, kt, C_BK_S, i)
            v_natural(wp, xbf, 2 * i)
            attn_core(i, True)
            o_proj_bn(i, wp, C_BNB_S, C_BNB_T)

            # ---- cross attention ----
            wp2 = wpool.tile([128, 4 * 2048], BF16, name="wp2", tag="wpack")
            nc.sync.dma_start(wp2[:], wcross_d[i])
            proj_T(wp2, 0, xbf, qt, C_BQ_X, i)
            proj_T(wp2, 1, encbf, kt, C_BK_X, i)
            v_natural(wp2, encbf, 2 * i + 1)
            attn_core(i, False)
            o_proj_bn(i, wp2, C_BNM_S, C_BNM_T)

            # ---- FFN ----
            wp3 = wpool.tile([128, 4 * 2048], BF16, name="wp3", tag="wpack")
            nc.sync.dma_start(wp3[:], wffn1_d[i])
            ht = [scr.tile([128, S], BF16, name=f"ht{fc}", tag="big") for fc in range(FT)]
            for fc in range(FT):
                for j in range(NJ):
                    ps = ps_mm.tile([128, 512], FP32, name="ps", tag="mm")
                    for k in range(KT):
                        nc.tensor.matmul(
                            ps[:],
                            wp3[:, k * 2048 + fc * 128:k * 2048 + (fc + 1) * 128],
                            xbf[k][:, j * 512:(j + 1) * 512],
                            start=(k == 0), stop=(k == KT - 1))
                    nc.scalar.activation(ht[fc][:, j * 512:(j + 1) * 512], ps[:],
                                         AF.Relu, bias=cc(i, C_B1, fc))
            wp4 = wpool.tile([128, 16 * 512], BF16, name="wp4", tag="wpack")
            nc.sync.dma_start(wp4[:], wffn2_d[i])
            for m in range(KT):
                for j in range(NJ):
                    ps = ps_mm.tile([128, 512], FP32, name="ps", tag="mm")
                    for k in range(FT):
                        nc.tensor.matmul(
                            ps[:],
                            wp4[:, k * 512 + m * 128:k * 512 + (m + 1) * 128],
                            ht[k][:, j * 512:(j + 1) * 512],
                            start=(k == 0), stop=(k == FT - 1))
                    xs = xt[m][:, j * 512:(j + 1) * 512]
                    nc.vector.tensor_add(xs, xs, ps[:])
                    nc.vector.tensor_scalar(xs, xs, cc(i, C_BNF_S, m),
                                            cc(i, C_BNF_T, m), ALU.mult, ALU.add)
                    nc.scalar.activation(xbf[m][:, j * 512:(j + 1) * 512], xs, AF.Copy)

        # ---- vocab projection ----
        for vc in range(VC):
            wg = wog.tile([128, 2000], BF16, name="wg", tag="wograb")
            nc.sync.dma_start(wg[:], wout_d[:, vc * 2000:(vc + 1) * 2000])
            for sc_i in range(SC):
                ps = ps_mm.tile([128, 512], FP32, name="ps", tag="mm")
                for k in range(KT):
                    nc.tensor.matmul(
                        ps[:, 0:500],
                        xbf[k][:, sc_i * 128:(sc_i + 1) * 128],
                        wg[:, k * 500:(k + 1) * 500],
                        start=(k == 0), stop=(k == KT - 1))
                osb = outp.tile([128, 500], FP32, name="osb", tag="osb")
                if sc_i % 2 == 0:
                    nc.vector.tensor_copy(osb[:], ps[:, 0:500])
                else:
                    nc.scalar.activation(osb[:], ps[:, 0:500], AF.Copy)
                nc.sync.dma_start(
                    logits_d[sc_i * 128:(sc_i + 1) * 128, vc * 500:(vc + 1) * 500],
                    osb[:])


def _prep_inputs(sequence, encoder_output, params):
    p = params
    embed = np.asarray(p["embed"], np.float32)
    pes = np.asarray(p["pes"], np.float32)
    seq = np.asarray(sequence)
    enc = np.asarray(encoder_output, np.float32)

    scale = 1.0 / np.sqrt(DH)

    wself = np.zeros((L, 128, 4 * 2048), ml_dtypes.bfloat16)
    wcross = np.zeros((L, 128, 4 * 2048), ml_dtypes.bfloat16)
    wffn1 = np.zeros((L, 128, 4 * 2048), ml_dtypes.bfloat16)
    wffn2 = np.zeros((L, 128, 16 * 512), ml_dtypes.bfloat16)
    consts = np.zeros((128, L * CL), np.float32)

    def put(i, base, vec):
        v = np.asarray(vec, np.float32).reshape(-1, 128)  # [n, 128] chunks? no:
        # vec is [n*128]; reshape to [n,128] then columns
        for m in range(v.shape[0]):
            consts[:, i * CL + base + m] = v[m]

    for i in range(L):
        bw, mw = p["bot"], p["mid"]
        wq_s = np.asarray(bw["Wq"][i], np.float32) * scale
        wself[i] = _bf(_pack_ktiles(np.concatenate(
            [wq_s, np.asarray(bw["Wk"][i], np.float32),
             np.asarray(bw["Wv"][i], np.float32),
             np.asarray(bw["Wo"][i], np.float32)], axis=1)))
        wq_x = np.asarray(mw["Wq"][i], np.float32) * scale
        wcross[i] = _bf(_pack_ktiles(np.concatenate(
            [wq_x, np.asarray(mw["Wk"][i], np.float32),
             np.asarray(mw["Wv"][i], np.float32),
             np.asarray(mw["Wo"][i], np.float32)], axis=1)))
        wffn1[i] = _bf(_pack_ktiles(np.asarray(p["ffn"]["W1"][i], np.float32)))
        wffn2[i] = _bf(_pack_ktiles(np.asarray(p["ffn"]["W2"][i], np.float32)))

        put(i, C_BQ_S, np.asarray(bw["bq"][i], np.float32) * scale)
        put(i, C_BK_S, bw["bk"][i])
        put(i, C_BV_S, bw["bv"][i])
        put(i, C_BQ_X, np.asarray(mw["bq"][i], np.float32) * scale)
        put(i, C_BK_X, mw["bk"][i])
        put(i, C_BV_X, mw["bv"][i])
        put(i, C_B1, p["ffn"]["b1"][i])

        for bn_name, s_base, t_base, extra in (
                ("bn_bot", C_BNB_S, C_BNB_T, np.asarray(p["bot"]["bo"][i], np.float32)),
                ("bn_mid", C_BNM_S, C_BNM_T, np.asarray(p["mid"]["bo"][i], np.float32)),
                ("bn_ffn", C_BNF_S, C_BNF_T, np.asarray(p["ffn"]["b2"][i], np.float32))):
            bn = p[bn_name]
            s_ = np.asarray(bn["gamma"][i], np.float32) / np.sqrt(
                np.asarray(bn["var"][i], np.float32) + EPS)
            t_ = np.asarray(bn["beta"][i], np.float32) - np.asarray(
                bn["mean"][i], np.float32) * s_ + extra * s_
            put(i, s_base, s_)
            put(i, t_base, t_)

    # per-(layer, attn) bv broadcast to all 128 partitions: [128, 512] blocks
    bvb = np.zeros((128, 2 * L * 512), ml_dtypes.bfloat16)
    for i in range(L):
        bvb[:, (2 * i) * 512:(2 * i + 1) * 512] = np.asarray(
            p["bot"]["bv"][i], np.float32)[None, :]
        bvb[:, (2 * i + 1) * 512:(2 * i + 2) * 512] = np.asarray(
            p["mid"]["bv"][i], np.float32)[None, :]

    # masks: M_r[k, q] = 1 if (k + 128*r) < q else 0, for 512-wide q chunks
    masks = np.zeros((128, 4 * 512), np.float32)
    for r in range(4):
        masks[:, r * 512:(r + 1) * 512] = (
            np.arange(128)[:, None] + 128 * r < np.arange(512)[None, :])
    masks = masks.astype(ml_dtypes.bfloat16)

    wout_halves = []
    wout_f = np.asarray(p["Wout"], np.float32)
    for half in range(2):
        wh = wout_f[:, half * VH:(half + 1) * VH]
        packed = np.zeros((128, VC * 2000), ml_dtypes.bfloat16)
        for vc in range(VC):
            for k in range(KT):
                packed[:, vc * 2000 + k * 500:vc * 2000 + (k + 1) * 500] = _bf(
                    wh[k * 128:(k + 1) * 128, vc * 500:(vc + 1) * 500])
        wout_halves.append(packed)

    in_maps = []
    for c in range(NCORES):
        b, half = c // 2, c % 2
        x0T = np.ascontiguousarray(
            (embed[seq[b]] + pes[:S]).T.astype(np.float32))
        encT = _bf(enc[b].T)
        in_maps.append({
            "x0T": x0T, "encT": np.ascontiguousarray(encT),
            "wself": wself, "wcross": wcross,
            "wffn1": wffn1, "wffn2": wffn2,
            "wout": wout_halves[half],
            "consts": consts, "masks": masks, "bvb": bvb,
        })
    return in_maps


def _run(sequence, encoder_output, params, trace=False):
    global _COMPILED
    if _COMPILED is None:
        _COMPILED = _build_program()
    nc = _COMPILED
    in_maps = _prep_inputs(sequence, encoder_output, params)
    res = bass_utils.run_bass_kernel_spmd(
        nc, in_maps, core_ids=list(range(NCORES)), trace=trace)

    bout = np.asarray(params["bout"], np.float32)
    out = np.empty((B, S, V), np.float32)
    for c in range(NCORES):
        b, half = c // 2, c % 2
        out[b, :, half * VH:(half + 1) * VH] = (
            res.results[c]["logits"] + bout[half * VH:(half + 1) * VH][None, :])
    return out, res


def kernel(sequence, encoder_output, params):
    out, _ = _run(sequence, encoder_output, params, trace=False)
    return out


# revision 12
# speedup vs baseline: 1.0163x; 1.0163x over previous
"""Trainium2 Bass kernel for a 4-layer transformer decoder (self+cross attn, FFN,
BatchNorm, vocab projection).

Sharding (hardcoded): 8 cores; core c handles batch b=c//2 and vocab half
v=c%2.  The 4-layer trunk is replicated within each core pair (batch-parallel
x4); the vocab projection is split column-wise x2 within the pair.  No
collectives.

Device-side layout: activations live in SBUF transposed as [feature, token]
(4 tiles of [128, 1024]).  All matmuls run in bf16 with fp32 PSUM
accumulation.  BatchNorm (inference affine) and the Wo/W2 output biases are
folded on the host into per-feature scale/shift vectors.  Softmax is computed
without max-subtraction (score magnitudes are small); row sums are obtained by
appending a ones-column to the V operand of the P@V matmul.
"""

import sys

for _p in ("/opt/trn_rl_repo", "/root/.axon_site/_ro/trn_rl_repo"):
    if _p not in sys.path:
        sys.path.append(_p)

import numpy as np
import ml_dtypes

import concourse.bass as bass
import concourse.mybir as mybir
import concourse.tile as tile
from concourse import bacc, bass_utils
from concourse.bass_interp import get_hw_module

FP32 = mybir.dt.float32
BF16 = mybir.dt.bfloat16
AF = mybir.ActivationFunctionType
ALU = mybir.AluOpType

V, D, L, H, S, SE, B = 32000, 512, 4, 8, 1024, 1024, 4
DH = D // H          # 64
NCORES = 8
VH = V // 2          # vocab half per core
EPS = 1e-3
KT = D // 128        # 4 k-tiles over D
FT = (4 * D) // 128  # 16 f-tiles over the FFN hidden dim
SC = S // 128        # 8 token chunks of 128
NJ = S // 512        # 2 token chunks of 512
VC = VH // 500       # 32 vocab chunks of 500

# consts column map (per layer, CL columns per layer)
CL = 64
C_BQ_S, C_BK_S, C_BV_S = 0, 4, 8
C_BQ_X, C_BK_X, C_BV_X = 12, 16, 20
C_B1 = 24
C_BNB_S, C_BNB_T = 40, 44
C_BNM_S, C_BNM_T = 48, 52
C_BNF_S, C_BNF_T = 56, 60

_COMPILED = None  # (nc_hw, meta)


def _bf(x):
    return np.ascontiguousarray(np.asarray(x, np.float32)).astype(ml_dtypes.bfloat16)


def _pack_ktiles(w):
    """[K, F] -> [128, (K//128)*F] with k-tile-major column blocks."""
    k = w.shape[0] // 128
    return np.concatenate([w[i * 128:(i + 1) * 128] for i in range(k)], axis=1)


def _build_program():
    nc = bacc.Bacc("TRN2", target_bir_lowering=False, debug=False,
                   enable_asserts=False, num_devices=NCORES)

    x0T_d = nc.dram_tensor("x0T", [D, S], FP32, kind="ExternalInput").ap()
    encT_d = nc.dram_tensor("encT", [D, SE], BF16, kind="ExternalInput").ap()
    wself_d = nc.dram_tensor("wself", [L, 128, 4 * 2048], BF16, kind="ExternalInput").ap()
    wcross_d = nc.dram_tensor("wcross", [L, 128, 4 * 2048], BF16, kind="ExternalInput").ap()
    wffn1_d = nc.dram_tensor("wffn1", [L, 128, 4 * 2048], BF16, kind="ExternalInput").ap()
    wffn2_d = nc.dram_tensor("wffn2", [L, 128, 16 * 512], BF16, kind="ExternalInput").ap()
    wout_d = nc.dram_tensor("wout", [128, VC * 2000], BF16, kind="ExternalInput").ap()
    consts_d = nc.dram_tensor("consts", [128, L * CL], FP32, kind="ExternalInput").ap()
    masks_d = nc.dram_tensor("masks", [128, 4 * 512], BF16, kind="ExternalInput").ap()
    bvb_d = nc.dram_tensor("bvb", [128, 2 * L * 512], BF16, kind="ExternalInput").ap()
    logits_d = nc.dram_tensor("logits", [S, VH], FP32, kind="ExternalOutput").ap()

    with tile.TileContext(nc) as tc:
        _emit(nc, tc, x0T_d, encT_d, wself_d, wcross_d, wffn1_d, wffn2_d,
              wout_d, consts_d, masks_d, bvb_d, logits_d)

    nc.compile()
    nc.m = get_hw_module(nc.m)
    return nc


def _emit(nc, tc, x0T_d, encT_d, wself_d, wcross_d, wffn1_d, wffn2_d,
          wout_d, consts_d, masks_d, bvb_d, logits_d):
    import contextlib
    ctx = contextlib.ExitStack()
    with ctx:
        persist = ctx.enter_context(tc.tile_pool(name="persist", bufs=1))
        wpool = ctx.enter_context(tc.tile_pool(name="wpool", bufs=3))
        scr = ctx.enter_context(tc.tile_pool(name="scr", bufs=20))
        smalls = ctx.enter_context(tc.tile_pool(name="smalls", bufs=2))
        outp = ctx.enter_context(tc.tile_pool(name="outp", bufs=6))
        wog = ctx.enter_context(tc.tile_pool(name="wog", bufs=5))
        ps_mm = ctx.enter_context(tc.tile_pool(name="ps_mm", bufs=2, space="PSUM"))
        ps_sc = ctx.enter_context(tc.tile_pool(name="ps_sc", bufs=2, space="PSUM"))
        ps_ap = ctx.enter_context(tc.tile_pool(name="ps_ap", bufs=2, space="PSUM"))

        # persistent tiles
        xt = [persist.tile([128, S], FP32, name=f"xt{m}", tag=f"xt{m}") for m in range(KT)]
        xbf = [persist.tile([128, S], BF16, name=f"xbf{m}", tag=f"xbf{m}") for m in range(KT)]
        encbf = [persist.tile([128, SE], BF16, name=f"encbf{m}", tag=f"encbf{m}") for m in range(KT)]
        qt = [persist.tile([128, S], BF16, name=f"qt{m}", tag=f"qt{m}") for m in range(KT)]
        kt = [persist.tile([128, SE], BF16, name=f"kt{m}", tag=f"kt{m}") for m in range(KT)]
        att = [persist.tile([128, S], BF16, name=f"att{m}", tag=f"att{m}") for m in range(KT)]
        vt = [persist.tile([128, 8 * 65], BF16, name=f"vt{t}", tag=f"vt{t}") for t in range(SC)]
        consts = persist.tile([128, L * CL], FP32, name="consts", tag="consts")
        maskt = persist.tile([128, 4 * 512], BF16, name="maskt", tag="maskt")
        bvb = persist.tile([128, 2 * L * 512], BF16, name="bvb", tag="bvb")
        ones1 = persist.tile([1, 64], FP32, name="ones1", tag="ones1")

        nc.sync.dma_start(consts[:], consts_d[:])
        nc.sync.dma_start(maskt[:], masks_d[:])
        nc.sync.dma_start(bvb[:], bvb_d[:])
        nc.vector.memset(ones1[:], 1.0)
        for m in range(KT):
            nc.sync.dma_start(xt[m][:], x0T_d[m * 128:(m + 1) * 128, :])
            nc.sync.dma_start(encbf[m][:], encT_d[m * 128:(m + 1) * 128, :])
            nc.scalar.activation(xbf[m][:], xt[m][:], AF.Copy)

        def cc(i, base, m):
            """consts column AP [128,1] for layer i."""
            return consts[:, i * CL + base + m : i * CL + base + m + 1]

        def proj_T(wp, widx, src, dst, bias_base, i):
            """dst[fm][:, :] (bf16, transposed layout) = W.T @ src + bias.
            wp: weight pack tile; widx: which 512-col weight in the pack;
            src: list of 4 bf16 [128, S] tiles."""
            for m in range(KT):
                for j in range(NJ):
                    ps = ps_mm.tile([128, 512], FP32, name="ps", tag="mm")
                    for k in range(KT):
                        nc.tensor.matmul(
                            ps[:],
                            wp[:, k * 2048 + widx * 512 + m * 128:
                               k * 2048 + widx * 512 + (m + 1) * 128],
                            src[k][:, j * 512:(j + 1) * 512],
                            start=(k == 0), stop=(k == KT - 1))
                    nc.vector.tensor_scalar(
                        dst[m][:, j * 512:(j + 1) * 512], ps[:],
                        cc(i, bias_base, m), None, ALU.add)

        def v_natural(wp, src, bv_blk):
            """vt[sc] heads-of-65 layout (bf16) = (src.T @ Wv + bv) per 128-token
            chunk, plus ones columns.  bv is folded here: P@(V+bv) = N + d*bv, so
            after the divide the attention output already carries +bv."""
            bv3 = bvb[:, bv_blk * 512:(bv_blk + 1) * 512].rearrange(
                "p (h c) -> p h c", h=H)
            for sc_i in range(SC):
                ps = ps_mm.tile([128, 512], FP32, name="ps", tag="mm")
                for k in range(KT):
                    nc.tensor.matmul(
                        ps[:],
                        src[k][:, sc_i * 128:(sc_i + 1) * 128],
                        wp[:, k * 2048 + 2 * 512:k * 2048 + 3 * 512],
                        start=(k == 0), stop=(k == KT - 1))
                v3 = vt[sc_i].rearrange("p (h c) -> p h c", h=H)
                p3 = ps.rearrange("p (h c) -> p h c", h=H)
                nc.vector.tensor_tensor(v3[:, :, 0:64], p3[:], bv3[:], ALU.add)
                nc.vector.memset(v3[:, :, 64:65], 1.0)

        def attn_core(i, causal):
            """scores/softmax/apply; writes att (bf16 transposed).  Heads are
            processed in pairs with adjacent score matmuls on partition bases
            0/64 so the PE packs them into disjoint row groups."""
            n_kt = SC
            for pr in range(H // 2):
                pt = {}   # (hh, t) -> P^T tile, hh in {0, 1}
                for t in range(n_kt):
                    # causal: for k-tile t only queries q > 128*t attend; restrict
                    # scores/exp to cols [128t, 1024) and zero-fill below.
                    lo = 128 * t if causal else 0
                    psc = [ps_sc.tile([128, 1024], FP32, name=f"psc{hh}", tag="sc")
                           for hh in range(2)]
                    for j in range(NJ):
                        a, b = max(j * 512, lo), (j + 1) * 512
                        if a >= b:
                            continue
                        for hh in range(2):
                            hp = 64 * hh
                            nc.tensor.matmul(
                                psc[hh][:, a:b],
                                kt[pr][hp:hp + 64, t * 128:(t + 1) * 128],
                                qt[pr][hp:hp + 64, a:b],
                                start=True, stop=True)
                    for hh in range(2):
                        p = scr.tile([128, 1024], BF16, name=f"pt{hh}", tag="big")
                        pt[(hh, t)] = p
                        nc.scalar.activation(p[:, lo:1024], psc[hh][:, lo:1024], AF.Exp)
                        if causal:
                            # zero-fill the skipped prefix of the 512-chunk that
                            # the apply matmul will read
                            j0 = lo // 512
                            if lo > j0 * 512:
                                nc.vector.memset(p[:, j0 * 512:lo], 0.0)
                            # diagonal mask: only the 128-col band q in
                            # [128t, 128t+128) is partial; beyond it the mask
                            # is all-ones, before it everything is zero-filled
                            r = t if t < 4 else t - 4
                            base = 0 if t < 4 else 512
                            c0 = base + 128 * r
                            m0 = r * 512 + 128 * r
                            nc.vector.tensor_tensor(
                                p[:, c0:c0 + 128], p[:, c0:c0 + 128],
                                maskt[:, m0:m0 + 128], ALU.mult)
                for hh in range(2):
                    h = 2 * pr + hh
                    hp = 64 * hh
                    for j in range(NJ):
                        ts_list = list(range(4)) if (causal and j == 0) else list(range(n_kt))
                        aps = ps_ap.tile([65, 512], FP32, name="aps", tag="ap")
                        for n, t in enumerate(ts_list):
                            nc.tensor.matmul(
                                aps[:],
                                vt[t][:, 65 * h:65 * h + 65],
                                pt[(hh, t)][:, j * 512:(j + 1) * 512],
                                start=(n == 0), stop=(n == len(ts_list) - 1))
                        if causal and j == 0:
                            nc.vector.memset(aps[64:65, 0:1], 1.0)
                        rc = smalls.tile([1, 512], FP32, name="rc", tag="rc")
                        nc.vector.reciprocal(rc[:], aps[64:65, :])
                        bcp = ps_mm.tile([64, 512], FP32, name="bcp", tag="mm")
                        nc.tensor.matmul(bcp[:], ones1[:, 0:64], rc[:],
                                         start=True, stop=True)
                        rb = smalls.tile([64, 512], FP32, name="rb", tag="rb")
                        nc.scalar.activation(rb[:], bcp[:], AF.Copy)
                        dst = att[pr][hp:hp + 64, j * 512:(j + 1) * 512]
                        nc.vector.tensor_tensor(dst, aps[0:64, :], rb[:], ALU.mult)

        def o_proj_bn(i, wp, s_base, t_base):
            """x = (x + Wo.T@att) * s + t'; refresh xbf."""
            for m in range(KT):
                for j in range(NJ):
                    ps = ps_mm.tile([128, 512], FP32, name="ps", tag="mm")
                    for k in range(KT):
                        nc.tensor.matmul(
                            ps[:],
                            wp[:, k * 2048 + 3 * 512 + m * 128:
                               k * 2048 + 3 * 512 + (m + 1) * 128],
                            att[k][:, j * 512:(j + 1) * 512],
                            start=(k == 0), stop=(k == KT - 1))
                    xs = xt[m][:, j * 512:(j + 1) * 512]
                    nc.vector.tensor_add(xs, xs, ps[:])
                    nc.vector.tensor_scalar(xs, xs, cc(i, s_base, m),
                                            cc(i, t_base, m), ALU.mult, ALU.add)
                    nc.scalar.activation(xbf[m][:, j * 512:(j + 1) * 512], xs, AF.Copy)

        for i in range(L):
            # ---- self attention ----
            wp = wpool.tile([128, 4 * 2048], BF16, name="wp", tag="wpack")
            nc.sync.dma_start(wp[:], wself_d[i])
            proj_T(wp, 0, xbf, qt, C_BQ_S, i)
            proj_T(wp, 1, xbf, kt, C_BK_S, i)
            v_natural(wp, xbf, 2 * i)
            attn_core(i, True)
            o_proj_bn(i, wp, C_BNB_S, C_BNB_T)

            # ---- cross attention ----
            wp2 = wpool.tile([128, 4 * 2048], BF16, name="wp2", tag="wpack")
            nc.sync.dma_start(wp2[:], wcross_d[i])
            proj_T(wp2, 0, xbf, qt, C_BQ_X, i)
            proj_T(wp2, 1, encbf, kt, C_BK_X, i)
            v_natural(wp2, encbf, 2 * i + 1)
            attn_core(i, False)
            o_proj_bn(i, wp2, C_BNM_S, C_BNM_T)

            # ---- FFN ----
            wp3 = wpool.tile([128, 4 * 2048], BF16, name="wp3", tag="wpack")
            nc.sync.dma_start(wp3[:], wffn1_d[i])
            ht = [scr.tile([128, S], BF16, name=f"ht{fc}", tag="big") for fc in range(FT)]
            for fc in range(FT):
                for j in range(NJ):
                    ps = ps_mm.tile([128, 512], FP32, name="ps", tag="mm")
                    for k in range(KT):
                        nc.tensor.matmul(
                            ps[:],
                            wp3[:, k * 2048 + fc * 128:k * 2048 + (fc + 1) * 128],
                            xbf[k][:, j * 512:(j + 1) * 512],
                            start=(k == 0), stop=(k == KT - 1))
                    nc.scalar.activation(ht[fc][:, j * 512:(j + 1) * 512], ps[:],
                                         AF.Relu, bias=cc(i, C_B1, fc))
            wp4 = wpool.tile([128, 16 * 512], BF16, name="wp4", tag="wpack")
            nc.sync.dma_start(wp4[:], wffn2_d[i])
            for m in range(KT):
                for j in range(NJ):
                    ps = ps_mm.tile([128, 512], FP32, name="ps", tag="mm")
                    for k in range(FT):
                        nc.tensor.matmul(
                            ps[:],
                            wp4[:, k * 512 + m * 128:k * 512 + (m + 1) * 128],
                            ht[k][:, j * 512:(j + 1) * 512],
                            start=(k == 0), stop=(k == FT - 1))
                    xs = xt[m][:, j * 512:(j + 1) * 512]
                    nc.vector.tensor_add(xs, xs, ps[:])
                    nc.vector.tensor_scalar(xs, xs, cc(i, C_BNF_S, m),
                                            cc(i, C_BNF_T, m), ALU.mult, ALU.add)
                    nc.scalar.activation(xbf[m][:, j * 512:(j + 1) * 512], xs, AF.Copy)

        # ---- vocab projection ----
        for vc in range(VC):
            wg = wog.tile([128, 2000], BF16, name="wg", tag="wograb")
            nc.sync.dma_start(wg[:], wout_d[:, vc * 2000:(vc + 1) * 2000])
            for sc_i in range(SC):
                ps = ps_mm.tile([128, 512], FP32, name="ps", tag="mm")
                for k in range(KT):
                    nc.tensor.matmul(
                        ps[:, 0:500],
                        xbf[k][:, sc_i * 128:(sc_i + 1) * 128],
                        wg[:, k * 500:(k + 1) * 500],
                        start=(k == 0), stop=(k == KT - 1))
                osb = outp.tile([128, 500], FP32, name="osb", tag="osb")
                if sc_i % 2 == 0:
                    nc.vector.tensor_copy(osb[:], ps[:, 0:500])
                else:
                    nc.scalar.activation(osb[:], ps[:, 0:500], AF.Copy)
                nc.sync.dma_start(
                    logits_d[sc_i * 128:(sc_i + 1) * 128, vc * 500:(vc + 1) * 500],
                    osb[:])


def _prep_inputs(sequence, encoder_output, params):
    p = params
    embed = np.asarray(p["embed"], np.float32)
    pes = np.asarray(p["pes"], np.float32)
    seq = np.asarray(sequence)
    enc = np.asarray(encoder_output, np.float32)

    scale = 1.0 / np.sqrt(DH)

    wself = np.zeros((L, 128, 4 * 2048), ml_dtypes.bfloat16)
    wcross = np.zeros((L, 128, 4 * 2048), ml_dtypes.bfloat16)
    wffn1 = np.zeros((L, 128, 4 * 2048), ml_dtypes.bfloat16)
    wffn2 = np.zeros((L, 128, 16 * 512), ml_dtypes.bfloat16)
    consts = np.zeros((128, L * CL), np.float32)

    def put(i, base, vec):
        v = np.asarray(vec, np.float32).reshape(-1, 128)  # [n, 128] chunks? no:
        # vec is [n*128]; reshape to [n,128] then columns
        for m in range(v.shape[0]):
            consts[:, i * CL + base + m] = v[m]

    for i in range(L):
        bw, mw = p["bot"], p["mid"]
        wq_s = np.asarray(bw["Wq"][i], np.float32) * scale
        wself[i] = _bf(_pack_ktiles(np.concatenate(
            [wq_s, np.asarray(bw["Wk"][i], np.float32),
             np.asarray(bw["Wv"][i], np.float32),
             np.asarray(bw["Wo"][i], np.float32)], axis=1)))
        wq_x = np.asarray(mw["Wq"][i], np.float32) * scale
        wcross[i] = _bf(_pack_ktiles(np.concatenate(
            [wq_x, np.asarray(mw["Wk"][i], np.float32),
             np.asarray(mw["Wv"][i], np.float32),
             np.asarray(mw["Wo"][i], np.float32)], axis=1)))
        wffn1[i] = _bf(_pack_ktiles(np.asarray(p["ffn"]["W1"][i], np.float32)))
        wffn2[i] = _bf(_pack_ktiles(np.asarray(p["ffn"]["W2"][i], np.float32)))

        put(i, C_BQ_S, np.asarray(bw["bq"][i], np.float32) * scale)
        put(i, C_BK_S, bw["bk"][i])
        put(i, C_BV_S, bw["bv"][i])
        put(i, C_BQ_X, np.asarray(mw["bq"][i], np.float32) * scale)
        put(i, C_BK_X, mw["bk"][i])
        put(i, C_BV_X, mw["bv"][i])
        put(i, C_B1, p["ffn"]["b1"][i])

        for bn_name, s_base, t_base, extra in (
                ("bn_bot", C_BNB_S, C_BNB_T, np.asarray(p["bot"]["bo"][i], np.float32)),
                ("bn_mid", C_BNM_S, C_BNM_T, np.asarray(p["mid"]["bo"][i], np.float32)),
                ("bn_ffn", C_BNF_S, C_BNF_T, np.asarray(p["ffn"]["b2"][i], np.float32))):
            bn = p[bn_name]
            s_ = np.asarray(bn["gamma"][i], np.float32) / np.sqrt(
                np.asarray(bn["var"][i], np.float32) + EPS)
            t_ = np.asarray(bn["beta"][i], np.float32) - np.asarray(
                bn["mean"][i], np.float32) * s_ + extra * s_
            put(i, s_base, s_)
            put(i, t_base, t_)

    # per-(layer, attn) bv broadcast to all 128 partitions: [128, 512] blocks
    bvb = np.zeros((128, 2 * L * 512), ml_dtypes.bfloat16)
    for i in range(L):
        bvb[:, (2 * i) * 512:(2 * i + 1) * 512] = np.asarray(
            p["bot"]["bv"][i], np.float32)[None, :]
        bvb[:, (2 * i + 1) * 512:(2 * i + 2) * 512] = np.asarray(
            p["mid"]["bv"][i], np.float32)[None, :]

    # masks: M_r[k, q] = 1 if (k + 128*r) < q else 0, for 512-wide q chunks
    masks = np.zeros((128, 4 * 512), np.float32)
    for r in range(4):
        masks[:, r * 512:(r + 1) * 512] = (
            np.arange(128)[:, None] + 128 * r < np.arange(512)[None, :])
    masks = masks.astype(ml_dtypes.bfloat16)

    wout_halves = []
    wout_f = np.asarray(p["Wout"], np.float32)
    for half in range(2):
        wh = wout_f[:, half * VH:(half + 1) * VH]
        packed = np.zeros((128, VC * 2000), ml_dtypes.bfloat16)
        for vc in range(VC):
            for k in range(KT):
                packed[:, vc * 2000 + k * 500:vc * 2000 + (k + 1) * 500] = _bf(
                    wh[k * 128:(k + 1) * 128, vc * 500:(vc + 1) * 500])
        wout_halves.append(packed)

    in_maps = []
    for c in range(NCORES):
        b, half = c // 2, c % 2
        x0T = np.ascontiguousarray(
            (embed[seq[b]] + pes[:S]).T.astype(np.float32))
        encT = _bf(enc[b].T)
        in_maps.append({
            "x0T": x0T, "encT": np.ascontiguousarray(encT),
            "wself": wself, "wcross": wcross,
            "wffn1": wffn1, "wffn2": wffn2,
            "wout": wout_halves[half],
            "consts": consts, "masks": masks, "bvb": bvb,
        })
    return in_maps


def _run(sequence, encoder_output, params, trace=False):
    global _COMPILED
    if _COMPILED is None:
        _COMPILED = _build_program()
    nc = _COMPILED
    in_maps = _prep_inputs(sequence, encoder_output, params)
    res = bass_utils.run_bass_kernel_spmd(
        nc, in_maps, core_ids=list(range(NCORES)), trace=trace)

    bout = np.asarray(params["bout"], np.float32)
    out = np.empty((B, S, V), np.float32)
    for c in range(NCORES):
        b, half = c // 2, c % 2
        out[b, :, half * VH:(half + 1) * VH] = (
            res.results[c]["logits"] + bout[half * VH:(half + 1) * VH][None, :])
    return out, res


def kernel(sequence, encoder_output, params):
    out, _ = _run(sequence, encoder_output, params, trace=False)
    return out
